# revision 17
# baseline (speedup 1.0000x reference)
"""Trainium2 Bass kernel for GaussianFPSPooling.

Two device phases to keep host<->device traffic minimal (the axon tunnel
moves ~30-60MB/s, so shipping the 205MB features tensor dominates any
single-phase design):

  Phase 1 (cores 0-3, data-parallel over batch B=4): farthest-point
    sampling over N=100000 3-D points, K=256 samples, fully SBUF-resident.
    Only the means (1.2MB/core) are uploaded; the index ramp and initial
    distance vector are generated on device (iota/memset).  Arithmetic
    replicates the jax-CPU reference bit-exactly ((x-px)^2 + (y-py)^2) +
    (z-pz)^2, f32, left-assoc, min accumulate, first-index argmax) so the
    selected indices match.  Returns the K indices per batch.

  Host: gathers the K=256 selected feature rows per batch and transposes
    them (data movement only, ~0.5MB).

  Phase 2 (core 0): sampled @ W + b for all 4 batches as 8 PE matmuls
    [128,128]x[128,256] with a rank-1 bias accumulate.
"""

import sys

if "/opt/trn_rl_repo" not in sys.path:
    sys.path.insert(0, "/opt/trn_rl_repo")

import numpy as np

import concourse.bacc as bacc
import concourse.bass as bass
import concourse.bass_isa as bass_isa
import concourse.mybir as mybir
from concourse import bass2jax, tile
from concourse.bass_utils import run_bass_kernel_spmd

try:
    # NEFF compiles of the unrolled FPS kernel take ~1 min; persist the jit
    # executables so fresh processes cold-start in <1s.
    import jax as _jax

    _jax.config.update("jax_compilation_cache_dir", "/tmp/jax_cc_cache")
    _jax.config.update("jax_persistent_cache_min_compile_time_secs", 0.5)
except Exception:
    pass

F32 = mybir.dt.float32
I32 = mybir.dt.int32
Alu = mybir.AluOpType
Act = mybir.ActivationFunctionType

# problem sizes (hardcoded per contract)
B = 4
N = 100000
D_IN = 128
D_OUT = 256
K = 256
P = 128               # partitions
BIGI = float(1 << 20)  # index-encoding base: stores BIGI - idx (exact in f32)

C = -(-N // P)        # cols per partition (782)
NPAD = P * C          # 100096


def build_fps_kernel():
    """Phase-1 program: FPS over one batch element.  In: xyzp [P, 3C+4]
    (xs | ys | zs | pt0 packed into one tensor to amortize the per-array
    transfer overhead of the axon tunnel).  Out: idx_out [1,K] (f32)."""
    nc = bacc.Bacc("TRN2", target_bir_lowering=False)

    xyzp_d = nc.dram_tensor("xyzp", [P, 3 * C + 4], F32, kind="ExternalInput")
    idx_d = nc.dram_tensor("idx_out", [1, K], F32, kind="ExternalOutput")

    with tile.TileContext(nc) as tc:
        with (
            tc.tile_pool(name="const", bufs=1) as cp,
            tc.tile_pool(name="loop", bufs=2) as lp,
        ):
            xyzp = cp.tile([P, 3 * C + 4], F32, tag="xyzp")
            xs = xyzp[:, 0:C]
            ys = xyzp[:, C:2 * C]
            zs = xyzp[:, 2 * C:3 * C]
            pt0 = xyzp[:, 3 * C:3 * C + 4]
            g2 = cp.tile([P, C], F32, tag="g2")
            g2i = cp.tile([P, C], I32, tag="g2i")
            dists = cp.tile([P, C], F32, tag="dists")
            idxraw = cp.tile([1, K], F32, tag="idxraw")

            nc.sync.dma_start(xyzp[:], xyzp_d[:])
            # g2[p][c] = BIGI - (p*C + c)  (the argmax tie-break encoding)
            nc.gpsimd.iota(g2i[:], pattern=[[-1, C]], base=1 << 20,
                           channel_multiplier=-C)
            nc.vector.tensor_copy(g2[:], g2i[:])
            # dists0 = +huge everywhere; the host fills padding slots with
            # point 0's coordinates, so their min-distance collapses to 0 on
            # the first iteration (point 0 is always sample 0) and padding
            # can never win the argmax.
            nc.vector.memset(dists[:], 1e30)
            nc.vector.memset(idxraw[:], BIGI)  # sample 0 is point 0

            pt = pt0
            for it in range(K - 1):
                px = pt[:, 0:1]
                py = pt[:, 1:2]
                pz = pt[:, 2:3]
                # d = ((x-px)^2 + (y-py)^2) + (z-pz)^2, bit-exact f32
                t1 = lp.tile([P, C], F32, tag="t1")
                nc.scalar.activation(t1[:], xs[:], Act.Square, bias=px, scale=-1.0)
                t2 = lp.tile([P, C], F32, tag="t2")
                nc.scalar.activation(t2[:], ys[:], Act.Square, bias=py, scale=-1.0)
                t3 = lp.tile([P, C], F32, tag="t3")
                nc.scalar.activation(t3[:], zs[:], Act.Square, bias=pz, scale=-1.0)
                s = lp.tile([P, C], F32, tag="s")
                nc.vector.tensor_tensor(s[:], t1[:], t2[:], op=Alu.add)
                nc.vector.tensor_tensor(s[:], s[:], t3[:], op=Alu.add)
                # dists = min(dists, d); permax = rowwise max of new dists
                permax = lp.tile([P, 1], F32, tag="permax")
                nc.vector.tensor_tensor(dists[:], dists[:], s[:], op=Alu.min)
                nc.vector.reduce_max(permax[:], dists[:], axis=mybir.AxisListType.X)
                gmax = lp.tile([P, 1], F32, tag="gmax")
                nc.gpsimd.partition_all_reduce(
                    gmax[:], permax[:], channels=P, reduce_op=bass_isa.ReduceOp.max
                )
                # encode argmax as max over (dists==gmax)*(BIGI-idx)
                mi = lp.tile([P, C], F32, tag="mi")
                nc.vector.scalar_tensor_tensor(
                    mi[:], in0=dists[:], scalar=gmax[:], in1=g2[:],
                    op0=Alu.is_equal, op1=Alu.mult,
                )
                permax2 = lp.tile([P, 1], F32, tag="permax2")
                nc.vector.reduce_max(permax2[:], mi[:], axis=mybir.AxisListType.X)
                is2 = lp.tile([P, 1], F32, tag="is2")
                nc.gpsimd.partition_all_reduce(
                    is2[:], permax2[:], channels=P, reduce_op=bass_isa.ReduceOp.max
                )
                # record BIGI - idx (decoded after the loop)
                nc.scalar.copy(idxraw[0:1, it + 1 : it + 2], is2[0:1, 0:1])
                # extract winner coords: one-hot (g2==is2) dot each plane
                ptn = lp.tile([P, 4], F32, tag="ptn")
                junk = lp.tile([P, C], F32, tag="junk")
                nc.vector.scalar_tensor_tensor(
                    junk[:], in0=g2[:], scalar=is2[:], in1=xs[:],
                    op0=Alu.is_equal, op1=Alu.mult, accum_out=ptn[:, 0:1],
                )
                nc.vector.scalar_tensor_tensor(
                    junk[:], in0=g2[:], scalar=is2[:], in1=ys[:],
                    op0=Alu.is_equal, op1=Alu.mult, accum_out=ptn[:, 1:2],
                )
                nc.vector.scalar_tensor_tensor(
                    junk[:], in0=g2[:], scalar=is2[:], in1=zs[:],
                    op0=Alu.is_equal, op1=Alu.mult, accum_out=ptn[:, 2:3],
                )
                ptb = lp.tile([P, 4], F32, tag="ptb")
                nc.gpsimd.partition_all_reduce(
                    ptb[:, 0:3], ptn[:, 0:3], channels=P,
                    reduce_op=bass_isa.ReduceOp.add,
                )
                pt = ptb

            # decode indices: idx = BIGI - idxraw
            idxf = cp.tile([1, K], F32, tag="idxf")
            nc.vector.tensor_scalar(
                idxf[:], idxraw[:], -1.0, BIGI, op0=Alu.mult, op1=Alu.add
            )
            nc.sync.dma_start(idx_d[:], idxf[:])

    nc.compile()
    return nc


def build_linear_kernel():
    """Phase-2 program: out[j*128:(j+1)*128] = sT[:, j*128:(j+1)*128].T @ W + b
    for j in 0..B*K//128, i.e. all four batches' Linear in one core.

    All inputs arrive in one packed tensor pk [P, B*K + D_OUT + 2]:
    sT (cols 0:B*K) | W (B*K:B*K+D_OUT) | bias packed as [P,2] (last 2 cols,
    relayouted to [1, D_OUT] by a strided DMA).  Output is bf16 (rel err
    ~2e-3 << the 2e-2 gate) to halve the D2H transfer."""
    cols = B * K                      # 1024
    nj = cols // P                    # 8 chunks
    BF16 = mybir.dt.bfloat16
    pkc = cols + D_OUT + 2            # 1282
    nc = bacc.Bacc("TRN2", target_bir_lowering=False)

    pk_d = nc.dram_tensor("pk", [P, pkc], F32, kind="ExternalInput")
    out_d = nc.dram_tensor("out", [cols, D_OUT], BF16, kind="ExternalOutput")

    with tile.TileContext(nc) as tc:
        with (
            tc.tile_pool(name="const", bufs=1) as cp,
            tc.tile_pool(name="loop", bufs=2) as lp,
            tc.tile_pool(name="psum", bufs=2, space="PSUM") as pp,
        ):
            pk = cp.tile([P, pkc], F32, tag="pk")
            st = pk[:, 0:cols]
            w_sb = pk[:, cols:cols + D_OUT]
            brow = cp.tile([1, D_OUT], F32, tag="brow")
            ones1 = cp.tile([1, P], F32, tag="ones1")
            nc.sync.dma_start(pk[:], pk_d[:])
            # bias relayout: DRAM [P,2] slice -> SBUF [1,256] in p-major order
            nc.sync.dma_start(brow[:], pk_d[:, cols + D_OUT:pkc])
            nc.vector.memset(ones1[:], 1.0)

            for j in range(nj):
                out_ps = pp.tile([P, D_OUT], F32, tag="outps")
                nc.tensor.matmul(
                    out_ps[:], lhsT=st[:, j * P : (j + 1) * P], rhs=w_sb[:],
                    start=True, stop=False,
                )
                nc.tensor.matmul(
                    out_ps[:], lhsT=ones1[:], rhs=brow[:], start=False, stop=True
                )
                outt = lp.tile([P, D_OUT], BF16, tag="outt")
                nc.vector.tensor_copy(outt[:], out_ps[:])
                nc.sync.dma_start(out_d[j * P : (j + 1) * P, :], outt[:])

    nc.compile()
    return nc


def make_fps_inputs(means_b):
    """Host-side layout of one batch element's means for phase 1: one packed
    [P, 3C+4] tensor = xs | ys | zs | pt0."""
    m = np.asarray(means_b, np.float32)
    planes = np.empty((NPAD, 3), np.float32)
    planes[:N] = m
    planes[N:] = m[0]  # pad = point 0 -> dist 0 after iter 1, never argmax
    xyzp = np.empty((P, 3 * C + 4), np.float32)
    xyzp[:, 0:C] = planes[:, 0].reshape(P, C)
    xyzp[:, C:2 * C] = planes[:, 1].reshape(P, C)
    xyzp[:, 2 * C:3 * C] = planes[:, 2].reshape(P, C)
    xyzp[:, 3 * C:3 * C + 3] = m[0]
    xyzp[:, 3 * C + 3] = 0.0
    return {"xyzp": xyzp}


_CACHE = {}


def _make_pjrt_runner(nc, n_cores):
    """Build the jit-compiled PJRT callable for `nc` ONCE and return a
    closure `run(in_maps) -> list[dict]`.

    run_bass_kernel_spmd constructs a fresh jax.jit closure on every call,
    which defeats the pjit cache and re-runs the full NEFF compile
    (walrus_driver) per invocation — several hundred ms each.  Hoisting the
    jit out of the per-call path makes repeat calls pure transfer+execute.
    Mirrors bass2jax.run_bass_via_pjrt.
    """
    import jax
    from jax.experimental.shard_map import shard_map
    from jax.sharding import Mesh, PartitionSpec

    bass2jax.install_neuronx_cc_hook()
    assert nc.dbg_addr is None or not nc.dbg_callbacks

    partition_name = nc.partition_id_tensor.name if nc.partition_id_tensor else None

    in_names, out_names, out_avals, zero_out_protos = [], [], [], []
    for alloc in nc.m.functions[0].allocations:
        if not isinstance(alloc, mybir.MemoryLocationSet):
            continue
        name = alloc.memorylocations[0].name
        if alloc.kind == "ExternalInput":
            if name != partition_name:
                in_names.append(name)
        elif alloc.kind == "ExternalOutput":
            shape = tuple(alloc.tensor_shape)
            dtype = mybir.dt.np(alloc.dtype)
            out_names.append(name)
            out_avals.append(jax.core.ShapedArray(shape, dtype))
            zero_out_protos.append((shape, dtype))
    n_params = len(in_names)
    n_outs = len(out_avals)
    all_in_names = list(in_names) + list(out_names)
    if partition_name is not None:
        all_in_names.append(partition_name)

    def _body(*args):
        operands = list(args)
        if partition_name is not None:
            operands.append(bass2jax.partition_id_tensor())
        outs = bass2jax._bass_exec_p.bind(
            *operands,
            out_avals=tuple(out_avals),
            in_names=tuple(all_in_names),
            out_names=tuple(out_names),
            lowering_input_output_aliases=(),
            sim_require_finite=True,
            sim_require_nnan=True,
            nc=nc,
        )
        return tuple(outs)

    dbg_extra = {}
    if nc.dbg_addr is not None:
        dbg_extra[nc.dbg_addr.name] = np.zeros((1, 2), np.uint32)

    # Our kernels write every element of every output, so the pre-zeroed
    # output operands are never read: keep ONE device-resident zero buffer
    # per output (no donation, so it stays valid) instead of shipping fresh
    # host zeros through the tunnel on every call.
    if n_cores == 1:
        jfn = jax.jit(_body, keep_unused=True)
        dev_zeros = [jax.device_put(np.zeros(s, d), jax.devices()[0])
                     for s, d in zero_out_protos]

        def run(in_maps):
            m = {**in_maps[0], **dbg_extra}
            ins = [np.asarray(m[name]) for name in in_names]
            out_arrs = jfn(*ins, *dev_zeros)
            return [{name: np.asarray(out_arrs[i])
                     for i, name in enumerate(out_names)}]

        return run

    devices = jax.devices()[:n_cores]
    mesh = Mesh(np.asarray(devices), ("core",))
    in_specs = (PartitionSpec("core"),) * (n_params + n_outs)
    out_specs = (PartitionSpec("core"),) * n_outs
    jfn = jax.jit(
        shard_map(_body, mesh=mesh, in_specs=in_specs, out_specs=out_specs,
                  check_rep=False),
        keep_unused=True,
    )
    from jax.sharding import NamedSharding
    dev_zeros = [
        jax.device_put(np.zeros((n_cores * s[0], *s[1:]), d),
                       NamedSharding(mesh, PartitionSpec("core")))
        for s, d in zero_out_protos
    ]

    def run(in_maps):
        maps = [{**m, **dbg_extra} for m in in_maps]
        concat_in = [
            np.concatenate([np.asarray(maps[c][name]) for c in range(n_cores)],
                           axis=0)
            for name in in_names
        ]
        out_arrs = jfn(*concat_in, *dev_zeros)
        return [
            {name: np.asarray(out_arrs[i]).reshape(n_cores, *out_avals[i].shape)[c]
             for i, name in enumerate(out_names)}
            for c in range(n_cores)
        ]

    return run


def _get_kernels():
    if "nc_fps" not in _CACHE:
        _CACHE["nc_fps"] = build_fps_kernel()
        _CACHE["nc_lin"] = build_linear_kernel()
        _CACHE["run_fps"] = _make_pjrt_runner(_CACHE["nc_fps"], B)
        _CACHE["run_lin"] = _make_pjrt_runner(_CACHE["nc_lin"], 1)
    return _CACHE["nc_fps"], _CACHE["nc_lin"]


def kernel(features, means, W, b, trace=False):
    import time as _time

    features = np.asarray(features, np.float32)
    means = np.asarray(means, np.float32)
    W = np.asarray(W, np.float32)
    b = np.asarray(b, np.float32)

    nc_fps, nc_lin = _get_kernels()

    t0 = _time.time()
    # --- phase 1: FPS on cores 0..B-1 (data-parallel over batch) ---
    in_maps = [make_fps_inputs(means[bb]) for bb in range(B)]
    res1 = _CACHE["run_fps"](in_maps)
    t1 = _time.time()
    idx = np.stack(
        [np.rint(res1[bb]["idx_out"][0]).astype(np.int64)
         for bb in range(B)], axis=0)                       # [B, K]

    # --- host: gather + transpose the selected rows (data movement) ---
    cols = B * K
    pk = np.empty((P, cols + D_OUT + 2), np.float32)
    for bb in range(B):
        pk[:, bb * K : (bb + 1) * K] = features[bb][idx[bb]].T
    pk[:, cols:cols + D_OUT] = W
    pk[:, cols + D_OUT:] = b.reshape(P, 2)
    t2 = _time.time()

    # --- phase 2: Linear on core 0 ---
    res2 = _CACHE["run_lin"]([{"pk": pk}])
    t3 = _time.time()
    _CACHE["phase_s"] = (t1 - t0, t2 - t1, t3 - t2)
    _CACHE["last_run_s"] = t3 - t0

    out_flat = np.asarray(res2[0]["out"], dtype=np.float32)  # [B*K, D_OUT]
    out = out_flat.reshape(B, K, D_OUT)
    _CACHE["last_results"] = None
    return out


if __name__ == "__main__":
    ins = dict(np.load("/tmp/inputs.npz"))
    out = kernel(**ins)
    print("out", out.shape, out.dtype)
